# revision 46
# baseline (speedup 1.0000x reference)
"""3-layer GCN + global mean pool + linear head on 8 Trainium2 NeuronCores.

Strategy (dst-sharded message passing, v3):
  - GCN normalization factorizes: norm_e = dinv[src]*dinv[dst], so each conv
    layer is  h' = relu( dinv * ((Adj+I) @ (dinv * h)) @ W + b ).  Only pure
    row gather + segment-sum on device; diagonal scalings are per-node ops.
  - Nodes (and their in-edges, self-loops appended) are sharded across the 8
    cores by contiguous dst ranges; 100 dst blocks of 128 slots per core.
  - The gather table (h~ rows, bf16, features padded to 128 -> 256B rows,
    the minimum dma_gather element) is split into 4 quarter-tables of 25,600
    rows so int16 gather indices reach every row.  Layer 1's table is
    precomputed on host (dinv*x) and passed as input - no collective.  For
    layers 2/3, each quarter's AllGather is issued as soon as its 25 blocks
    finish the previous layer's epilogue, overlapping wire time with the
    gather/matmul pipeline of the current layer.
  - Edges are grouped by (dst block, src quarter); chunk counts are padded
    to the cross-core max so the SPMD program is uniform.  dma_gather emits
    descriptors only for real+dummy tokens: each (super-group, quarter)
    call's trailing padding uses index -1, which SWDGE skips (no
    descriptor, no bytes, num_idxs_reg = emitted count).  Stale SBUF in
    those slots multiplies against all-zero one-hot columns (dst_rel=-1),
    contributing exactly zero (msgs buffers are zeroed on first use so the
    stale data is always finite).  Calls are <=1024 tokens (SWDGE ring
    cap), round-robined over 4 SWDGE queues for DMA-ring parallelism.
  - Segment-sum runs on the TensorEngine: per 128-edge chunk,
    aggT[64f,128d] += msgs[128e,0:64].T @ M[128e,128d], with M built by one
    broadcast is_equal against an iota tile.  The layer weight applies
    after aggregation (W commutes with the sum), then dinv/bias/relu on the
    DVE.  Mean-pool one-hot matmuls are folded into the layer-3 epilogue;
    partials are AllReduced and the head matmul finishes on every core.

Host-side work is sharding-style preprocessing only: edge sort/group/pad,
degree bincount (dinv), graph-size bincount, layout permutation.
"""
import numpy as np
import ml_dtypes

P = 128
NCORES = 8
NQ = 4            # quarter tables (int16 source windows)
QB = 25           # dst blocks per quarter
NBLK = NQ * QB    # dst blocks per core
SGB = 5           # dst blocks per super-group (msgs buffer granularity)
MSGS_BUFS = 3     # msgs tile pool depth
TMAX = 1024       # max tokens per dma_gather call (SWDGE ring limit)
NQUEUE = 4        # SWDGE queues (ucode MAX_SWDGE_QUEUES)

# Full-size problem dims (nn_GCN_13881334300836)
N_FULL, E_FULL, D_FULL, C_FULL, G_FULL = 100_000, 1_250_000, 64, 10, 128


# --------------------------------------------------------------------------
# Host preprocessing
# --------------------------------------------------------------------------

def preprocess(x, edge_index, batch, n_cores=NCORES):
    """Shard nodes/edges; build quarter-grouped, chunk-padded gather indices.

    Node n -> (core c = n//npc, local i): block b = i//128, slot p = i%128,
    quarter q = b//QB.  Table row within quarter q: (c*128 + p)*QB + b%QB.
    Each core's h~ quarter-slice is one contiguous SBUF->DRAM DMA and the
    AllGather concatenation over cores reproduces this row layout.
    """
    N, H = x.shape
    assert N % n_cores == 0
    npc = N // n_cores
    nblk = NBLK
    assert nblk * P >= npc
    qrows = n_cores * P * QB          # rows per quarter table
    assert qrows <= 32768

    ei = edge_index.astype(np.int64)
    loop = np.arange(N, dtype=np.int64)
    src_all = np.concatenate([ei[0], loop])
    dst_all = np.concatenate([ei[1], loop])

    deg = np.bincount(dst_all, minlength=N).astype(np.float32)
    dinv = (1.0 / np.sqrt(np.maximum(deg, 1.0))).astype(np.float32)

    core_of = src_all // npc
    local = src_all - core_of * npc
    b_src = local // P
    p_src = local % P
    q_src = b_src // QB
    row_in_q = (core_of * P + p_src) * QB + (b_src % QB)

    # per-core (block, quarter) edge lists
    edges = []   # [core][b][q] -> (idx int16 array, drel array)
    cnt_all = np.zeros((n_cores, nblk, NQ), np.int64)
    for c in range(n_cores):
        lo = c * npc
        m = (dst_all >= lo) & (dst_all < lo + npc)
        r = row_in_q[m]
        q = q_src[m]
        d = dst_all[m] - lo
        key = (d // P) * NQ + q
        order = np.argsort(key, kind="stable")
        r, d, key = r[order], d[order], key[order]
        cnt = np.bincount(key, minlength=nblk * NQ)
        off = np.zeros(nblk * NQ + 1, np.int64)
        np.cumsum(cnt, out=off[1:])
        percore = []
        for b in range(nblk):
            row = []
            for qq in range(NQ):
                k = b * NQ + qq
                sl = slice(off[k], off[k + 1])
                row.append((r[sl].astype(np.int16),
                            (d[sl] % P).astype(np.float32)))
                cnt_all[c, b, qq] = cnt[k]
            percore.append(row)
        edges.append(percore)

    U = cnt_all.max(axis=0)                    # [nblk, NQ] uniform counts
    Kb = (U + P - 1) // P                      # chunks per group
    nsg = nblk // SGB

    # chunk stream: for sg: for q: for b in sg (max-pad group last)
    chunk_pos = np.zeros((nblk, NQ), np.int64)
    sg_tok0 = []
    calls = []        # [sg] -> list of (q, tok_a, tok_b, reg_cnt)
    tails = []        # [sg] -> list of (chunk_lo, chunk_hi) never-gathered
    emit_len = np.zeros((nblk, NQ), np.int64)  # tokens emitted per group
    pos = 0
    for sg in range(nsg):
        sg_tok0.append(pos * P)
        sg_calls = []
        sg_tails = []
        for q in range(NQ):
            blocks = [b for b in range(sg * SGB, (sg + 1) * SGB)
                      if Kb[b, q] > 0]
            if not blocks:
                continue
            blocks.sort(key=lambda b: Kb[b, q] * P - U[b, q])
            t0 = pos * P
            for i, b in enumerate(blocks):
                chunk_pos[b, q] = pos
                pos += Kb[b, q]
                emit_len[b, q] = (U[b, q] if i == len(blocks) - 1
                                  else Kb[b, q] * P)
            t1 = pos * P
            # emitted tokens end at tail_start; -1 tail after that
            tail_start = t1 - (Kb[blocks[-1], q] * P - U[blocks[-1], q])
            if tail_start < t1:
                sg_tails.append((tail_start // P, t1 // P))
            a = t0
            while a < t1:
                bnd = min(a + TMAX, t1)
                reg = max(0, min(bnd, tail_start) - a)
                if reg > 0:
                    sg_calls.append((q, a, bnd, reg))
                a = bnd
        calls.append(sg_calls)
        tails.append(sg_tails)
    nchunk = pos
    ntok = nchunk * P

    eidx16 = np.zeros((n_cores, 16, ntok // 16), np.int16)
    edst = np.full((n_cores, P, nchunk), -1.0, np.float32)
    for c in range(n_cores):
        stream = np.full(ntok, -1, np.int16)
        for b in range(nblk):
            for q in range(NQ):
                r16, dr = edges[c][b][q]
                t0 = chunk_pos[b, q] * P
                L = emit_len[b, q]
                seg = np.zeros(L, np.int16)        # dummy idx 0 padding
                seg[:len(r16)] = r16
                stream[t0:t0 + L] = seg
                kb = Kb[b, q]
                dcols = np.full((kb * P,), -1.0, np.float32)
                dcols[:len(dr)] = dr
                edst[c][:, chunk_pos[b, q]:chunk_pos[b, q] + kb] = \
                    dcols.reshape(kb, P).T
        eidx16[c] = stream.reshape(ntok // 16, 16).T

    npad = nblk * P
    dinv_pc = np.zeros((n_cores, P, nblk), np.float32)
    bat_pc = np.full((n_cores, P, nblk), -1.0, np.float32)
    for c in range(n_cores):
        dv = np.zeros(npad, np.float32)
        dv[:npc] = dinv[c * npc:(c + 1) * npc]
        dinv_pc[c] = dv.reshape(nblk, P).T
        bt = np.full(npad, -1.0, np.float32)
        bt[:npc] = batch[c * npc:(c + 1) * npc].astype(np.float32)
        bat_pc[c] = bt.reshape(nblk, P).T

    # layer-1 gather table: x~ = dinv * x, bf16, 256B rows, quarter layout
    EL = P
    xf = np.asarray(x, np.float32) * dinv[:, None]
    xq = np.zeros((NQ, n_cores * P, QB * EL), np.float32)
    for c in range(n_cores):
        xp = np.zeros((npad, H), np.float32)
        xp[:npc] = xf[c * npc:(c + 1) * npc]
        # slot (b, p) -> quarter b//QB, row c*128+p, cols (b%QB)*EL..+H
        xp4 = xp.reshape(NQ, QB, P, H)
        for q in range(NQ):
            blk = np.zeros((P, QB, EL), np.float32)
            blk[:, :, :H] = xp4[q].transpose(1, 0, 2)
            xq[q, c * P:(c + 1) * P, :] = blk.reshape(P, QB * EL)

    return dict(eidx16=eidx16, edst=edst, dinv=dinv_pc, batg=bat_pc, xq=xq,
                npc=npc, nblk=nblk, nsg=nsg, ntok=ntok, nchunk=nchunk,
                Kb=Kb, chunk_pos=chunk_pos, sg_tok0=sg_tok0, calls=calls,
                tails=tails, qrows=qrows, H=H)


# --------------------------------------------------------------------------
# Device kernel builder
# --------------------------------------------------------------------------

def build_nc(pp, G, C, n_cores=NCORES, ablate=()):
    """Build the Bass program (shared SPMD across n_cores)."""
    import concourse.bacc as bacc
    import concourse.mybir as mybir
    import concourse.tile as tile
    from contextlib import ExitStack

    H = pp["H"]
    nblk, nsg, ntok, nchunk = pp["nblk"], pp["nsg"], pp["ntok"], pp["nchunk"]
    Kb, chunk_pos = pp["Kb"], pp["chunk_pos"]
    sg_tok0, calls, tails = pp["sg_tok0"], pp["calls"], pp["tails"]
    RG = [list(range(n_cores))]
    EL = P  # padded feature width (256B rows)

    f32, bf16 = mybir.dt.float32, mybir.dt.bfloat16
    i16 = mybir.dt.int16
    AL = mybir.AluOpType

    nc = bacc.Bacc("TRN2", target_bir_lowering=False, debug=False,
                   enable_asserts=False, num_devices=n_cores,
                   num_swdge_queues=NQUEUE)

    eidx_d = nc.dram_tensor("eidx", [16, ntok // 16], i16, kind="ExternalInput")
    edst_d = nc.dram_tensor("edst", [P, nchunk], bf16, kind="ExternalInput")
    xq_d = [nc.dram_tensor(f"xq{q}", [n_cores * P, QB * EL], bf16,
                           kind="ExternalInput") for q in range(NQ)]
    dinv_d = nc.dram_tensor("dinv", [P, nblk], f32, kind="ExternalInput")
    batg_d = nc.dram_tensor("batg", [P, nblk], f32, kind="ExternalInput")
    iota_bf_d = nc.dram_tensor("iota_bf", [P, P], bf16, kind="ExternalInput")
    iota_f_d = nc.dram_tensor("iota_f", [P, P], f32, kind="ExternalInput")
    w_d = [nc.dram_tensor(f"w{l}", [H, H], f32, kind="ExternalInput")
           for l in range(3)]
    bias_d = [nc.dram_tensor(f"bias{l}", [P, H], f32, kind="ExternalInput")
              for l in range(3)]
    wl_d = nc.dram_tensor("wl", [H, C], f32, kind="ExternalInput")
    biasl_d = nc.dram_tensor("biasl", [P, C], f32, kind="ExternalInput")
    cinv_d = nc.dram_tensor("cinv", [P, 1], f32, kind="ExternalInput")
    out_d = nc.dram_tensor("out", [G, C], f32, kind="ExternalOutput")

    with tile.TileContext(nc) as tc:
        with ExitStack() as ctx:
            const = ctx.enter_context(tc.tile_pool(name="const", bufs=1))
            mb_bufs = 4 if "mbufs4" in ablate else MSGS_BUFS
            msgs_tp = ctx.enter_context(tc.tile_pool(name="msgs",
                                                     bufs=mb_bufs))
            m_tp = ctx.enter_context(tc.tile_pool(name="mb", bufs=3))
            s_tp = ctx.enter_context(tc.tile_pool(name="st", bufs=3))
            e_tp = ctx.enter_context(tc.tile_pool(name="ep", bufs=4))
            agg_ps = ctx.enter_context(tc.tile_pool(
                name="aggp", bufs=5 if "agg5" in ablate else 4,
                space="PSUM"))
            out_ps = ctx.enter_context(tc.tile_pool(name="outp", bufs=2,
                                                    space="PSUM"))
            fin_ps = ctx.enter_context(tc.tile_pool(name="finp", bufs=1,
                                                    space="PSUM"))
            dram = ctx.enter_context(tc.tile_pool(name="dram", bufs=1,
                                                  space="DRAM"))

            eidx_sb = const.tile([128, ntok // 16], i16)
            edst_sb = const.tile([P, nchunk], bf16)
            iota_bf = const.tile([P, P], bf16)
            iota_f = const.tile([P, P], f32)
            dinv_sb = const.tile([P, nblk], f32)
            batg_sb = const.tile([P, nblk], f32)
            w_sb = [const.tile([H, H], f32, tag=f"w{l}", name=f"w{l}_sb")
                    for l in range(3)]
            bias_sb = [const.tile([P, H], f32, tag=f"b{l}", name=f"b{l}_sb")
                       for l in range(3)]
            wl_sb = const.tile([H, C], f32)
            biasl_sb = const.tile([P, C], f32)
            cinv_sb = const.tile([P, 1], f32)
            ht_sb = const.tile([P, nblk, EL], bf16)   # h~ slice, 256B rows
            h3_sb = const.tile([P, nblk * H], f32)

            # idx tile: replicate the [16, S] wrap to all 8 partition groups
            for g8 in range(8):
                nc.sync.dma_start(eidx_sb[:][g8 * 16:(g8 + 1) * 16, :],
                                  eidx_d.ap())
            nc.sync.dma_start(edst_sb[:], edst_d.ap())
            nc.sync.dma_start(iota_bf[:], iota_bf_d.ap())
            nc.sync.dma_start(iota_f[:], iota_f_d.ap())
            nc.sync.dma_start(dinv_sb[:], dinv_d.ap())
            nc.sync.dma_start(batg_sb[:], batg_d.ap())
            for l in range(3):
                nc.sync.dma_start(w_sb[l][:], w_d[l].ap())
                nc.sync.dma_start(bias_sb[l][:], bias_d[l].ap())
            nc.sync.dma_start(wl_sb[:], wl_d.ap())
            nc.sync.dma_start(biasl_sb[:], biasl_d.ap())
            nc.sync.dma_start(cinv_sb[:], cinv_d.ap())
            # zero the padding feature columns of h~ (and empty blocks) once
            nc.vector.memset(ht_sb[:], 0.0)
            nc.vector.memset(h3_sb[:], 0.0)

            mbc = None
            if "mb" in ablate:
                mbc = const.tile([P, 16 * P], bf16, tag="mbc")
                nc.vector.memset(mbc[:], 0.0)
            msgs_c = None
            if "gather" in ablate:
                sg_max = max(
                    (sg_tok0[sg + 1] if sg + 1 < nsg else ntok) - sg_tok0[sg]
                    for sg in range(nsg))
                msgs_c = const.tile([P, sg_max // P, EL], bf16, tag="msgsc")
                nc.vector.memset(msgs_c[:], 0.0)

            # compact collective buffers (128B rows) + expanded gather tables
            in_cc = [[dram.tile([P, QB * H], bf16, tag=f"incc{l}{q}",
                                name=f"incc{l}{q}") for q in range(NQ)]
                     for l in range(2)]
            hqc = [[dram.tile([n_cores * P, QB * H], bf16,
                              addr_space="Shared", tag=f"hqc{l}{q}",
                              name=f"hqc{l}{q}") for q in range(NQ)]
                   for l in range(2)]
            hq = [[dram.tile([n_cores * P, QB * EL], bf16,
                             addr_space=("Shared" if "compactag" not in ablate
                                         else "Local"),
                             tag=f"hq{l}{q}", name=f"hq{l}{q}")
                   for q in range(NQ)] for l in range(2)]
            in_ccF = None
            if "compactag" not in ablate:
                in_ccF = [[dram.tile([P, QB * EL], bf16, tag=f"incf{l}{q}",
                                     name=f"incf{l}{q}") for q in range(NQ)]
                          for l in range(2)]
            exp_in_tp = ctx.enter_context(tc.tile_pool(name="expin", bufs=2))
            exp_out_tp = ctx.enter_context(tc.tile_pool(name="expout",
                                                        bufs=2))
            prd_in = dram.tile([H, P], f32)
            prd_out = dram.tile([H, P], f32, addr_space="Shared")

            def table_ap(l, q):
                if l == 0:
                    return xq_d[q].ap().rearrange("p (b e) -> (p b) e", e=EL)
                return hq[l - 1][q][:].rearrange("p (b e) -> (p b) e", e=EL)

            def expand_piece(l, qd, pc, first):
                """One 128-row piece of the 128B->256B table expansion."""
                ei_t = exp_in_tp.tile([P, QB * H], bf16, tag="ei", name="ei")
                eo_t = exp_out_tp.tile([P, QB * EL], bf16, tag="eo",
                                       name="eo")
                if first:
                    nc.vector.memset(eo_t[:], 0.0)
                eng = nc.scalar if pc % 2 else nc.sync
                eng.dma_start(ei_t[:], hqc[l][qd][pc * P:(pc + 1) * P, :])
                nc.vector.tensor_copy(
                    out=eo_t[:].rearrange("p (b e) -> p b e",
                                          e=EL)[:, :, 0:H],
                    in_=ei_t[:].rearrange("p (b e) -> p b e", e=H))
                eng.dma_start(hq[l][qd][pc * P:(pc + 1) * P, :], eo_t[:])

            # piece (qd, pc) runs during super-group 5*qd+5+... : one full
            # quarter of compute after its AllGather was issued (guard
            # against head-blocking the in-order HWDGE queues)
            exp_sched = {}
            spread = [5, 5, 6, 6, 7, 8, 8, 9]
            for qd in range(NQ):
                for pc in range(n_cores):
                    exp_sched.setdefault(5 * qd + spread[pc],
                                         []).append((qd, pc))

            nonempty = [b for b in range(nblk) if Kb[b].sum() > 0]
            poolT = fin_ps.tile([H, P], f32, tag="poolT")
            call_no = 0
            sg_cmax = max(
                ((sg_tok0[sg + 1] if sg + 1 < nsg else ntok) - sg_tok0[sg])
                // P for sg in range(nsg))
            for l in range(3):
                last = l == 2
                for sg in range(nsg):
                    tok0 = sg_tok0[sg]
                    tok1 = sg_tok0[sg + 1] if sg + 1 < nsg else ntok
                    sg_ntok = tok1 - tok0
                    msgs = (msgs_c if "gather" in ablate else
                            msgs_tp.tile([P, sg_cmax, EL], bf16,
                                         tag="msgs", name="msgs"))
                    if "gather" not in ablate:
                        # First use of each physical msgs buffer: zero it all
                        # (stale SBUF may decode NaN; NaN*0=NaN through the
                        # one-hot matmul).  Afterwards never-gathered tail
                        # slots hold finite leftovers, and finite*0 = 0.
                        if l == 0 and sg < mb_bufs:
                            nc.vector.memset(msgs[:], 0.0)
                        for (q, a, bnd, reg) in calls[sg]:
                            nc.gpsimd.dma_gather(
                                out_ap=msgs[:][:, (a - tok0) // P:
                                               (bnd - tok0) // P, :],
                                in_ap=table_ap(l, q),
                                idxs_ap=eidx_sb[:][:, a // 16:bnd // 16],
                                num_idxs=bnd - a, num_idxs_reg=reg,
                                elem_size=EL,
                                single_packet="sp0" not in ablate,
                                queue_num=call_no % (2 if "q2" in ablate
                                                     else NQUEUE))
                            call_no += 1
                    for bi in range(sg * SGB, (sg + 1) * SGB):
                        nmm = int(Kb[bi].sum())
                        if nmm == 0:
                            continue  # empty block: ht/h3 stay zero
                        aggT = agg_ps.tile([H, P], f32, tag="agg", name="agg")
                        imm = 0
                        for q in range(NQ):
                            kb = int(Kb[bi, q])
                            if kb == 0:
                                continue
                            col = int(chunk_pos[bi, q])
                            MB = m_tp.tile([P, kb * P], bf16, tag="MB",
                                           name="MB")
                            if "mb" not in ablate:
                                nc.vector.tensor_tensor(
                                    out=MB[:].rearrange("p (c q) -> p c q",
                                                        q=P),
                                    in0=edst_sb[:][:, col:col + kb]
                                        .to_broadcast([P, kb, P]),
                                    in1=iota_bf[:][:, None, :]
                                        .to_broadcast([P, kb, P]),
                                    op=AL.is_equal)
                            for j in range(kb):
                                mc = col + j - tok0 // P
                                rhs = (mbc[:][:, (j % 16) * P:(j % 16 + 1) * P]
                                       if "mb" in ablate
                                       else MB[:][:, j * P:(j + 1) * P])
                                if ("mm" not in ablate or imm == 0
                                        or imm == nmm - 1):
                                    nc.tensor.matmul(
                                        aggT[:],
                                        lhsT=msgs[:][:, mc, 0:H],
                                        rhs=rhs,
                                        start=(imm == 0),
                                        stop=(imm == nmm - 1))
                                imm += 1
                        sT = s_tp.tile([H, P], f32, tag="sT", name="sT")
                        nc.scalar.copy(out=sT[:], in_=aggT[:])
                        outb = out_ps.tile([P, H], f32, tag="outb",
                                           name="outb")
                        nc.tensor.matmul(outb[:], lhsT=sT[:], rhs=w_sb[l][:],
                                         start=True, stop=True)
                        dcol = dinv_sb[:][:, bi:bi + 1]
                        t1_ = e_tp.tile([P, H], f32, tag="t1", name="t1")
                        nc.vector.tensor_scalar(
                            out=t1_[:], in0=outb[:], scalar1=dcol,
                            scalar2=None, op0=AL.mult)
                        if not last:
                            t2 = e_tp.tile([P, H], f32, tag="t2", name="t2")
                            nc.vector.tensor_tensor(
                                out=t2[:], in0=t1_[:], in1=bias_sb[l][:],
                                op=AL.add)
                            nc.vector.tensor_scalar(
                                out=ht_sb[:][:, bi, 0:H], in0=t2[:],
                                scalar1=0.0, scalar2=dcol,
                                op0=AL.max, op1=AL.mult)
                        else:
                            nc.vector.tensor_tensor(
                                out=h3_sb[:][:, bi * H:(bi + 1) * H],
                                in0=t1_[:], in1=bias_sb[l][:], op=AL.add)
                            # pooling: poolT[f,g] += h3_b[n,f]*(batch[n]==g)
                            Mg = m_tp.tile([P, P], f32, tag="Mg", name="Mg")
                            nc.vector.tensor_scalar(
                                out=Mg[:], in0=iota_f[:],
                                scalar1=batg_sb[:][:, bi:bi + 1],
                                scalar2=None, op0=AL.is_equal)
                            nc.tensor.matmul(
                                poolT[:],
                                lhsT=h3_sb[:][:, bi * H:(bi + 1) * H],
                                rhs=Mg[:], start=(bi == nonempty[0]),
                                stop=(bi == nonempty[-1]))
                    if not last and (sg + 1) % (QB // SGB) == 0:
                        qd = sg // (QB // SGB)
                        if "compactag" not in ablate:
                            nc.sync.dma_start(
                                in_ccF[l][qd][:],
                                ht_sb[:][:, qd * QB:(qd + 1) * QB, :]
                                    .rearrange("p b e -> p (b e)"))
                            if "coll" not in ablate:
                                nc.gpsimd.collective_compute(
                                    "AllGather", AL.bypass,
                                    replica_groups=RG,
                                    ins=[in_ccF[l][qd].opt()],
                                    outs=[hq[l][qd].opt()])
                            continue
                        nc.sync.dma_start(
                            in_cc[l][qd][:].rearrange("p (b e) -> p b e",
                                                      e=H),
                            ht_sb[:][:, qd * QB:(qd + 1) * QB, 0:H])
                        if "coll" not in ablate:
                            nc.gpsimd.collective_compute(
                                "AllGather", AL.bypass, replica_groups=RG,
                                ins=[in_cc[l][qd].opt()],
                                outs=[hqc[l][qd].opt()])
                    if (not last and "coll" not in ablate
                            and "compactag" in ablate):
                        for (qd, pc) in exp_sched.get(sg, []):
                            expand_piece(l, qd, pc,
                                         l == 0 and qd == 0 and pc < 2)
                if (not last and "coll" not in ablate
                        and "compactag" in ablate):
                    for sgv in range(nsg, nsg + SGB):
                        for (qd, pc) in exp_sched.get(sgv, []):
                            expand_piece(l, qd, pc, False)


            poolT_sb = s_tp.tile([H, P], f32, tag="poolTs")
            nc.vector.tensor_copy(out=poolT_sb[:], in_=poolT[:])
            nc.sync.dma_start(prd_in[:], poolT_sb[:])
            nc.gpsimd.collective_compute(
                "AllReduce", AL.add, replica_groups=RG,
                ins=[prd_in.opt()], outs=[prd_out.opt()])
            poolF = s_tp.tile([H, P], f32, tag="poolF")
            nc.sync.dma_start(poolF[:], prd_out[:])
            fin = fin_ps.tile([P, C], f32, tag="fin")
            nc.tensor.matmul(fin[:], lhsT=poolF[:], rhs=wl_sb[:],
                             start=True, stop=True)
            outf = e_tp.tile([P, C], f32, tag="outf")
            nc.vector.tensor_scalar(out=outf[:], in0=fin[:],
                                    scalar1=cinv_sb[:], scalar2=None,
                                    op0=AL.mult)
            outf2 = e_tp.tile([P, C], f32, tag="outf2")
            nc.vector.tensor_tensor(out=outf2[:], in0=outf[:],
                                    in1=biasl_sb[:], op=AL.add)
            nc.sync.dma_start(out_d.ap()[:, :], outf2[:][:G, :])

    nc.compile()
    return nc


def make_in_maps(pp, weights, G, n_cores=NCORES):
    W1, b1, W2, b2, W3, b3, Wl, bl, counts = weights
    H = pp["H"]
    C = np.asarray(Wl).shape[1]
    bf = ml_dtypes.bfloat16
    iota_row = np.arange(P, dtype=np.float32)
    iota_bf = np.ascontiguousarray(np.broadcast_to(iota_row, (P, P))).astype(bf)
    iota_f = np.ascontiguousarray(np.broadcast_to(iota_row, (P, P)))
    cinv = np.ones((P, 1), np.float32)
    cinv[:G, 0] = 1.0 / np.maximum(counts, 1.0)
    shared = {
        "iota_bf": iota_bf, "iota_f": iota_f,
        "w0": np.asarray(W1, np.float32), "w1": np.asarray(W2, np.float32),
        "w2": np.asarray(W3, np.float32),
        "bias0": np.ascontiguousarray(np.broadcast_to(b1, (P, H))).astype(np.float32),
        "bias1": np.ascontiguousarray(np.broadcast_to(b2, (P, H))).astype(np.float32),
        "bias2": np.ascontiguousarray(np.broadcast_to(b3, (P, H))).astype(np.float32),
        "wl": np.asarray(Wl, np.float32),
        "biasl": np.ascontiguousarray(np.broadcast_to(bl, (P, C))).astype(np.float32),
        "cinv": cinv,
    }
    for q in range(NQ):
        shared[f"xq{q}"] = pp["xq"][q].astype(bf)
    maps = []
    for c in range(n_cores):
        m = dict(shared)
        m["eidx"] = pp["eidx16"][c]
        m["edst"] = pp["edst"][c].astype(bf)
        m["dinv"] = pp["dinv"][c]
        m["batg"] = pp["batg"][c]
        maps.append(m)
    return maps


LAST_RESULT = None
LAST_NC = None
LAST_IN_MAPS = None


def kernel(x, edge_index, batch, W1, b1, W2, b2, W3, b3, Wl, bl, **run_kwargs):
    """Full-input entry point. Shards across 8 cores, runs on HW, gathers."""
    global LAST_RESULT, LAST_NC, LAST_IN_MAPS
    from concourse.bass_utils import run_bass_kernel_spmd

    x = np.asarray(x, np.float32)
    edge_index = np.asarray(edge_index)
    batch = np.asarray(batch)
    G = G_FULL
    C = np.asarray(Wl).shape[1]

    pp = preprocess(x, edge_index, batch)
    counts = np.bincount(batch.astype(np.int64), minlength=G).astype(np.float32)
    nc = build_nc(pp, G, C)
    in_maps = make_in_maps(pp, (W1, b1, W2, b2, W3, b3, Wl, bl, counts), G)
    res = run_bass_kernel_spmd(nc, in_maps, core_ids=list(range(NCORES)),
                               **run_kwargs)
    LAST_RESULT, LAST_NC, LAST_IN_MAPS = res, nc, in_maps
    return res.results[0]["out"].astype(np.float32)


# revision 47
# speedup vs baseline: 1.0605x; 1.0605x over previous
"""3-layer GCN + global mean pool + linear head on 8 Trainium2 NeuronCores.

Strategy (dst-sharded message passing, v3):
  - GCN normalization factorizes: norm_e = dinv[src]*dinv[dst], so each conv
    layer is  h' = relu( dinv * ((Adj+I) @ (dinv * h)) @ W + b ).  Only pure
    row gather + segment-sum on device; diagonal scalings are per-node ops.
  - Nodes (and their in-edges, self-loops appended) are sharded across the 8
    cores by contiguous dst ranges; 100 dst blocks of 128 slots per core.
  - The gather table (h~ rows, bf16, features padded to 128 -> 256B rows,
    the minimum dma_gather element) is split into 4 quarter-tables of 25,600
    rows so int16 gather indices reach every row.  Layer 1's table is
    precomputed on host (dinv*x) and passed as input - no collective.  For
    layers 2/3, each quarter's AllGather is issued as soon as its 25 blocks
    finish the previous layer's epilogue, overlapping wire time with the
    gather/matmul pipeline of the current layer.
  - Edges are grouped by (dst block, src quarter); chunk counts are padded
    to the cross-core max so the SPMD program is uniform.  dma_gather emits
    descriptors only for real+dummy tokens: each (super-group, quarter)
    call's trailing padding uses index -1, which SWDGE skips (no
    descriptor, no bytes, num_idxs_reg = emitted count).  Stale SBUF in
    those slots multiplies against all-zero one-hot columns (dst_rel=-1),
    contributing exactly zero (msgs buffers are zeroed on first use so the
    stale data is always finite).  Calls are <=1024 tokens (SWDGE ring
    cap), round-robined over 4 SWDGE queues for DMA-ring parallelism.
  - Segment-sum runs on the TensorEngine: per 128-edge chunk,
    aggT[64f,128d] += msgs[128e,0:64].T @ M[128e,128d], with M built by one
    broadcast is_equal against an iota tile.  The layer weight applies
    after aggregation (W commutes with the sum), then dinv/bias/relu on the
    DVE.  Mean-pool one-hot matmuls are folded into the layer-3 epilogue;
    partials are AllReduced and the head matmul finishes on every core.

Host-side work is sharding-style preprocessing only: edge sort/group/pad,
degree bincount (dinv), graph-size bincount, layout permutation.
"""
import numpy as np
import ml_dtypes

P = 128
NCORES = 8
NQ = 4            # quarter tables (int16 source windows)
QB = 25           # dst blocks per quarter
NBLK = NQ * QB    # dst blocks per core
SGB = 5           # dst blocks per super-group (msgs buffer granularity)
MSGS_BUFS = 3     # msgs tile pool depth
TMAX = 1024       # max tokens per dma_gather call (SWDGE ring limit)
NQUEUE = 4        # SWDGE queues (ucode MAX_SWDGE_QUEUES)

# Full-size problem dims (nn_GCN_13881334300836)
N_FULL, E_FULL, D_FULL, C_FULL, G_FULL = 100_000, 1_250_000, 64, 10, 128


# --------------------------------------------------------------------------
# Host preprocessing
# --------------------------------------------------------------------------

def preprocess(x, edge_index, batch, n_cores=NCORES):
    """Shard nodes/edges; build quarter-grouped, chunk-padded gather indices.

    Node n -> (core c = n//npc, local i): block b = i//128, slot p = i%128,
    quarter q = b//QB.  Table row within quarter q: (c*128 + p)*QB + b%QB.
    Each core's h~ quarter-slice is one contiguous SBUF->DRAM DMA and the
    AllGather concatenation over cores reproduces this row layout.
    """
    N, H = x.shape
    assert N % n_cores == 0
    npc = N // n_cores
    nblk = NBLK
    assert nblk * P >= npc
    qrows = n_cores * P * QB          # rows per quarter table
    assert qrows <= 32768

    ei = edge_index.astype(np.int64)
    loop = np.arange(N, dtype=np.int64)
    src_all = np.concatenate([ei[0], loop])
    dst_all = np.concatenate([ei[1], loop])

    deg = np.bincount(dst_all, minlength=N).astype(np.float32)
    dinv = (1.0 / np.sqrt(np.maximum(deg, 1.0))).astype(np.float32)

    core_of = src_all // npc
    local = src_all - core_of * npc
    b_src = local // P
    p_src = local % P
    q_src = b_src // QB
    row_in_q = (core_of * P + p_src) * QB + (b_src % QB)

    # per-core (block, quarter) edge lists
    edges = []   # [core][b][q] -> (idx int16 array, drel array)
    cnt_all = np.zeros((n_cores, nblk, NQ), np.int64)
    for c in range(n_cores):
        lo = c * npc
        m = (dst_all >= lo) & (dst_all < lo + npc)
        r = row_in_q[m]
        q = q_src[m]
        d = dst_all[m] - lo
        key = (d // P) * NQ + q
        order = np.argsort(key, kind="stable")
        r, d, key = r[order], d[order], key[order]
        cnt = np.bincount(key, minlength=nblk * NQ)
        off = np.zeros(nblk * NQ + 1, np.int64)
        np.cumsum(cnt, out=off[1:])
        percore = []
        for b in range(nblk):
            row = []
            for qq in range(NQ):
                k = b * NQ + qq
                sl = slice(off[k], off[k + 1])
                row.append((r[sl].astype(np.int16),
                            (d[sl] % P).astype(np.float32)))
                cnt_all[c, b, qq] = cnt[k]
            percore.append(row)
        edges.append(percore)

    U = cnt_all.max(axis=0)                    # [nblk, NQ] uniform counts
    Kb = (U + P - 1) // P                      # chunks per group
    nsg = nblk // SGB

    # chunk stream: for sg: for q: for b in sg (max-pad group last)
    chunk_pos = np.zeros((nblk, NQ), np.int64)
    sg_tok0 = []
    calls = []        # [sg] -> list of (q, tok_a, tok_b, reg_cnt)
    tails = []        # [sg] -> list of (chunk_lo, chunk_hi) never-gathered
    emit_len = np.zeros((nblk, NQ), np.int64)  # tokens emitted per group
    pos = 0
    for sg in range(nsg):
        sg_tok0.append(pos * P)
        sg_calls = []
        sg_tails = []
        for q in range(NQ):
            blocks = [b for b in range(sg * SGB, (sg + 1) * SGB)
                      if Kb[b, q] > 0]
            if not blocks:
                continue
            blocks.sort(key=lambda b: Kb[b, q] * P - U[b, q])
            t0 = pos * P
            for i, b in enumerate(blocks):
                chunk_pos[b, q] = pos
                pos += Kb[b, q]
                emit_len[b, q] = (U[b, q] if i == len(blocks) - 1
                                  else Kb[b, q] * P)
            t1 = pos * P
            # emitted tokens end at tail_start; -1 tail after that
            tail_start = t1 - (Kb[blocks[-1], q] * P - U[blocks[-1], q])
            if tail_start < t1:
                sg_tails.append((tail_start // P, t1 // P))
            a = t0
            while a < t1:
                bnd = min(a + TMAX, t1)
                reg = max(0, min(bnd, tail_start) - a)
                if reg > 0:
                    sg_calls.append((q, a, bnd, reg))
                a = bnd
        calls.append(sg_calls)
        tails.append(sg_tails)
    nchunk = pos
    ntok = nchunk * P

    eidx16 = np.zeros((n_cores, 16, ntok // 16), np.int16)
    edst = np.full((n_cores, P, nchunk), -1.0, np.float32)
    for c in range(n_cores):
        stream = np.full(ntok, -1, np.int16)
        for b in range(nblk):
            for q in range(NQ):
                r16, dr = edges[c][b][q]
                t0 = chunk_pos[b, q] * P
                L = emit_len[b, q]
                seg = np.zeros(L, np.int16)        # dummy idx 0 padding
                seg[:len(r16)] = r16
                stream[t0:t0 + L] = seg
                kb = Kb[b, q]
                dcols = np.full((kb * P,), -1.0, np.float32)
                dcols[:len(dr)] = dr
                edst[c][:, chunk_pos[b, q]:chunk_pos[b, q] + kb] = \
                    dcols.reshape(kb, P).T
        eidx16[c] = stream.reshape(ntok // 16, 16).T

    npad = nblk * P
    dinv_pc = np.zeros((n_cores, P, nblk), np.float32)
    bat_pc = np.full((n_cores, P, nblk), -1.0, np.float32)
    for c in range(n_cores):
        dv = np.zeros(npad, np.float32)
        dv[:npc] = dinv[c * npc:(c + 1) * npc]
        dinv_pc[c] = dv.reshape(nblk, P).T
        bt = np.full(npad, -1.0, np.float32)
        bt[:npc] = batch[c * npc:(c + 1) * npc].astype(np.float32)
        bat_pc[c] = bt.reshape(nblk, P).T

    # layer-1 gather table: x~ = dinv * x, bf16, 256B rows, quarter layout
    EL = P
    xf = np.asarray(x, np.float32) * dinv[:, None]
    xq = np.zeros((NQ, n_cores * P, QB * EL), np.float32)
    for c in range(n_cores):
        xp = np.zeros((npad, H), np.float32)
        xp[:npc] = xf[c * npc:(c + 1) * npc]
        # slot (b, p) -> quarter b//QB, row c*128+p, cols (b%QB)*EL..+H
        xp4 = xp.reshape(NQ, QB, P, H)
        for q in range(NQ):
            blk = np.zeros((P, QB, EL), np.float32)
            blk[:, :, :H] = xp4[q].transpose(1, 0, 2)
            xq[q, c * P:(c + 1) * P, :] = blk.reshape(P, QB * EL)

    return dict(eidx16=eidx16, edst=edst, dinv=dinv_pc, batg=bat_pc, xq=xq,
                npc=npc, nblk=nblk, nsg=nsg, ntok=ntok, nchunk=nchunk,
                Kb=Kb, chunk_pos=chunk_pos, sg_tok0=sg_tok0, calls=calls,
                tails=tails, qrows=qrows, H=H)


# --------------------------------------------------------------------------
# Device kernel builder
# --------------------------------------------------------------------------

def build_nc(pp, G, C, n_cores=NCORES, ablate=()):
    """Build the Bass program (shared SPMD across n_cores)."""
    import concourse.bacc as bacc
    import concourse.mybir as mybir
    import concourse.tile as tile
    from contextlib import ExitStack

    H = pp["H"]
    nblk, nsg, ntok, nchunk = pp["nblk"], pp["nsg"], pp["ntok"], pp["nchunk"]
    Kb, chunk_pos = pp["Kb"], pp["chunk_pos"]
    sg_tok0, calls, tails = pp["sg_tok0"], pp["calls"], pp["tails"]
    RG = [list(range(n_cores))]
    EL = P  # padded feature width (256B rows)

    f32, bf16 = mybir.dt.float32, mybir.dt.bfloat16
    i16 = mybir.dt.int16
    AL = mybir.AluOpType

    nc = bacc.Bacc("TRN2", target_bir_lowering=False, debug=False,
                   enable_asserts=False, num_devices=n_cores,
                   num_swdge_queues=NQUEUE)

    eidx_d = nc.dram_tensor("eidx", [16, ntok // 16], i16, kind="ExternalInput")
    edst_d = nc.dram_tensor("edst", [P, nchunk], bf16, kind="ExternalInput")
    xq_d = [nc.dram_tensor(f"xq{q}", [n_cores * P, QB * EL], bf16,
                           kind="ExternalInput") for q in range(NQ)]
    dinv_d = nc.dram_tensor("dinv", [P, nblk], f32, kind="ExternalInput")
    batg_d = nc.dram_tensor("batg", [P, nblk], f32, kind="ExternalInput")
    iota_bf_d = nc.dram_tensor("iota_bf", [P, P], bf16, kind="ExternalInput")
    iota_f_d = nc.dram_tensor("iota_f", [P, P], f32, kind="ExternalInput")
    w_d = [nc.dram_tensor(f"w{l}", [H, H], f32, kind="ExternalInput")
           for l in range(3)]
    bias_d = [nc.dram_tensor(f"bias{l}", [P, H], f32, kind="ExternalInput")
              for l in range(3)]
    wl_d = nc.dram_tensor("wl", [H, C], f32, kind="ExternalInput")
    biasl_d = nc.dram_tensor("biasl", [P, C], f32, kind="ExternalInput")
    cinv_d = nc.dram_tensor("cinv", [P, 1], f32, kind="ExternalInput")
    out_d = nc.dram_tensor("out", [G, C], f32, kind="ExternalOutput")

    with tile.TileContext(nc) as tc:
        with ExitStack() as ctx:
            const = ctx.enter_context(tc.tile_pool(name="const", bufs=1))
            mb_bufs = 4 if "mbufs4" in ablate else MSGS_BUFS
            msgs_tp = ctx.enter_context(tc.tile_pool(name="msgs",
                                                     bufs=mb_bufs))
            m_tp = ctx.enter_context(tc.tile_pool(name="mb", bufs=3))
            s_tp = ctx.enter_context(tc.tile_pool(name="st", bufs=3))
            e_tp = ctx.enter_context(tc.tile_pool(name="ep", bufs=4))
            agg_ps = ctx.enter_context(tc.tile_pool(name="aggp", bufs=4,
                                                    space="PSUM"))
            out_ps = ctx.enter_context(tc.tile_pool(name="outp", bufs=2,
                                                    space="PSUM"))
            fin_ps = ctx.enter_context(tc.tile_pool(name="finp", bufs=1,
                                                    space="PSUM"))
            dram = ctx.enter_context(tc.tile_pool(name="dram", bufs=1,
                                                  space="DRAM"))

            eidx_sb = const.tile([128, ntok // 16], i16)
            edst_sb = const.tile([P, nchunk], bf16)
            iota_bf = const.tile([P, P], bf16)
            iota_f = const.tile([P, P], f32)
            dinv_sb = const.tile([P, nblk], f32)
            batg_sb = const.tile([P, nblk], f32)
            w_sb = [const.tile([H, H], f32, tag=f"w{l}", name=f"w{l}_sb")
                    for l in range(3)]
            bias_sb = [const.tile([P, H], f32, tag=f"b{l}", name=f"b{l}_sb")
                       for l in range(3)]
            wl_sb = const.tile([H, C], f32)
            biasl_sb = const.tile([P, C], f32)
            cinv_sb = const.tile([P, 1], f32)
            ht_sb = const.tile([P, nblk, EL], bf16)   # h~ slice, 256B rows
            h3_sb = const.tile([P, nblk * H], f32)

            # idx tile: replicate the [16, S] wrap to all 8 partition groups
            for g8 in range(8):
                nc.sync.dma_start(eidx_sb[:][g8 * 16:(g8 + 1) * 16, :],
                                  eidx_d.ap())
            nc.sync.dma_start(edst_sb[:], edst_d.ap())
            nc.sync.dma_start(iota_bf[:], iota_bf_d.ap())
            nc.sync.dma_start(iota_f[:], iota_f_d.ap())
            nc.sync.dma_start(dinv_sb[:], dinv_d.ap())
            nc.sync.dma_start(batg_sb[:], batg_d.ap())
            for l in range(3):
                nc.sync.dma_start(w_sb[l][:], w_d[l].ap())
                nc.sync.dma_start(bias_sb[l][:], bias_d[l].ap())
            nc.sync.dma_start(wl_sb[:], wl_d.ap())
            nc.sync.dma_start(biasl_sb[:], biasl_d.ap())
            nc.sync.dma_start(cinv_sb[:], cinv_d.ap())
            # zero the padding feature columns of h~ (and empty blocks) once
            nc.vector.memset(ht_sb[:], 0.0)
            nc.vector.memset(h3_sb[:], 0.0)

            mbc = None
            if "mb" in ablate:
                mbc = const.tile([P, 16 * P], bf16, tag="mbc")
                nc.vector.memset(mbc[:], 0.0)
            msgs_c = None
            if "gather" in ablate:
                sg_max = max(
                    (sg_tok0[sg + 1] if sg + 1 < nsg else ntok) - sg_tok0[sg]
                    for sg in range(nsg))
                msgs_c = const.tile([P, sg_max // P, EL], bf16, tag="msgsc")
                nc.vector.memset(msgs_c[:], 0.0)

            # compact collective buffers (128B rows) + expanded gather tables
            in_cc = [[dram.tile([P, QB * H], bf16, tag=f"incc{l}{q}",
                                name=f"incc{l}{q}") for q in range(NQ)]
                     for l in range(2)]
            hqc = [[dram.tile([n_cores * P, QB * H], bf16,
                              addr_space="Shared", tag=f"hqc{l}{q}",
                              name=f"hqc{l}{q}") for q in range(NQ)]
                   for l in range(2)]
            hq = [[dram.tile([n_cores * P, QB * EL], bf16,
                             addr_space=("Shared" if "compactag" not in ablate
                                         else "Local"),
                             tag=f"hq{l}{q}", name=f"hq{l}{q}")
                   for q in range(NQ)] for l in range(2)]
            in_ccF = None
            if "compactag" not in ablate:
                in_ccF = [[dram.tile([P, QB * EL], bf16, tag=f"incf{l}{q}",
                                     name=f"incf{l}{q}") for q in range(NQ)]
                          for l in range(2)]
            exp_in_tp = ctx.enter_context(tc.tile_pool(name="expin", bufs=2))
            exp_out_tp = ctx.enter_context(tc.tile_pool(name="expout",
                                                        bufs=2))
            prd_in = dram.tile([H, P], f32)
            prd_out = dram.tile([H, P], f32, addr_space="Shared")

            def table_ap(l, q):
                if l == 0:
                    return xq_d[q].ap().rearrange("p (b e) -> (p b) e", e=EL)
                return hq[l - 1][q][:].rearrange("p (b e) -> (p b) e", e=EL)

            def expand_piece(l, qd, pc, first):
                """One 128-row piece of the 128B->256B table expansion."""
                ei_t = exp_in_tp.tile([P, QB * H], bf16, tag="ei", name="ei")
                eo_t = exp_out_tp.tile([P, QB * EL], bf16, tag="eo",
                                       name="eo")
                if first:
                    nc.vector.memset(eo_t[:], 0.0)
                eng = nc.scalar if pc % 2 else nc.sync
                eng.dma_start(ei_t[:], hqc[l][qd][pc * P:(pc + 1) * P, :])
                nc.vector.tensor_copy(
                    out=eo_t[:].rearrange("p (b e) -> p b e",
                                          e=EL)[:, :, 0:H],
                    in_=ei_t[:].rearrange("p (b e) -> p b e", e=H))
                eng.dma_start(hq[l][qd][pc * P:(pc + 1) * P, :], eo_t[:])

            # piece (qd, pc) runs during super-group 5*qd+5+... : one full
            # quarter of compute after its AllGather was issued (guard
            # against head-blocking the in-order HWDGE queues)
            exp_sched = {}
            spread = [5, 5, 6, 6, 7, 8, 8, 9]
            for qd in range(NQ):
                for pc in range(n_cores):
                    exp_sched.setdefault(5 * qd + spread[pc],
                                         []).append((qd, pc))

            nonempty = [b for b in range(nblk) if Kb[b].sum() > 0]
            poolT = fin_ps.tile([H, P], f32, tag="poolT")
            call_no = 0
            sg_cmax = max(
                ((sg_tok0[sg + 1] if sg + 1 < nsg else ntok) - sg_tok0[sg])
                // P for sg in range(nsg))
            for l in range(3):
                last = l == 2
                for sg in range(nsg):
                    tok0 = sg_tok0[sg]
                    tok1 = sg_tok0[sg + 1] if sg + 1 < nsg else ntok
                    sg_ntok = tok1 - tok0
                    msgs = (msgs_c if "gather" in ablate else
                            msgs_tp.tile([P, sg_cmax, EL], bf16,
                                         tag="msgs", name="msgs"))
                    if "gather" not in ablate:
                        # First use of each physical msgs buffer: zero it all
                        # (stale SBUF may decode NaN; NaN*0=NaN through the
                        # one-hot matmul).  Afterwards never-gathered tail
                        # slots hold finite leftovers, and finite*0 = 0.
                        if l == 0 and sg < mb_bufs:
                            nc.vector.memset(msgs[:], 0.0)
                        for (q, a, bnd, reg) in calls[sg]:
                            nc.gpsimd.dma_gather(
                                out_ap=msgs[:][:, (a - tok0) // P:
                                               (bnd - tok0) // P, :],
                                in_ap=table_ap(l, q),
                                idxs_ap=eidx_sb[:][:, a // 16:bnd // 16],
                                num_idxs=bnd - a, num_idxs_reg=reg,
                                elem_size=EL,
                                single_packet="sp0" not in ablate,
                                queue_num=call_no % (2 if "q2" in ablate
                                                     else NQUEUE))
                            call_no += 1
                    for bi in range(sg * SGB, (sg + 1) * SGB):
                        nmm = int(Kb[bi].sum())
                        if nmm == 0:
                            continue  # empty block: ht/h3 stay zero
                        aggT = agg_ps.tile([H, P], f32, tag="agg", name="agg")
                        imm = 0
                        for q in range(NQ):
                            kb = int(Kb[bi, q])
                            if kb == 0:
                                continue
                            col = int(chunk_pos[bi, q])
                            MB = m_tp.tile([P, kb * P], bf16, tag="MB",
                                           name="MB")
                            if "mb" not in ablate:
                                nc.vector.tensor_tensor(
                                    out=MB[:].rearrange("p (c q) -> p c q",
                                                        q=P),
                                    in0=edst_sb[:][:, col:col + kb]
                                        .to_broadcast([P, kb, P]),
                                    in1=iota_bf[:][:, None, :]
                                        .to_broadcast([P, kb, P]),
                                    op=AL.is_equal)
                            for j in range(kb):
                                mc = col + j - tok0 // P
                                rhs = (mbc[:][:, (j % 16) * P:(j % 16 + 1) * P]
                                       if "mb" in ablate
                                       else MB[:][:, j * P:(j + 1) * P])
                                if ("mm" not in ablate or imm == 0
                                        or imm == nmm - 1):
                                    nc.tensor.matmul(
                                        aggT[:],
                                        lhsT=msgs[:][:, mc, 0:H],
                                        rhs=rhs,
                                        start=(imm == 0),
                                        stop=(imm == nmm - 1))
                                imm += 1
                        sT = s_tp.tile([H, P], f32, tag="sT", name="sT")
                        nc.scalar.copy(out=sT[:], in_=aggT[:])
                        outb = out_ps.tile([P, H], f32, tag="outb",
                                           name="outb")
                        nc.tensor.matmul(outb[:], lhsT=sT[:], rhs=w_sb[l][:],
                                         start=True, stop=True)
                        dcol = dinv_sb[:][:, bi:bi + 1]
                        t1_ = e_tp.tile([P, H], f32, tag="t1", name="t1")
                        nc.vector.tensor_scalar(
                            out=t1_[:], in0=outb[:], scalar1=dcol,
                            scalar2=None, op0=AL.mult)
                        if not last:
                            t2 = e_tp.tile([P, H], f32, tag="t2", name="t2")
                            nc.vector.tensor_tensor(
                                out=t2[:], in0=t1_[:], in1=bias_sb[l][:],
                                op=AL.add)
                            nc.vector.tensor_scalar(
                                out=ht_sb[:][:, bi, 0:H], in0=t2[:],
                                scalar1=0.0, scalar2=dcol,
                                op0=AL.max, op1=AL.mult)
                        else:
                            nc.vector.tensor_tensor(
                                out=h3_sb[:][:, bi * H:(bi + 1) * H],
                                in0=t1_[:], in1=bias_sb[l][:], op=AL.add)
                            # pooling: poolT[f,g] += h3_b[n,f]*(batch[n]==g)
                            Mg = m_tp.tile([P, P], f32, tag="Mg", name="Mg")
                            nc.vector.tensor_scalar(
                                out=Mg[:], in0=iota_f[:],
                                scalar1=batg_sb[:][:, bi:bi + 1],
                                scalar2=None, op0=AL.is_equal)
                            nc.tensor.matmul(
                                poolT[:],
                                lhsT=h3_sb[:][:, bi * H:(bi + 1) * H],
                                rhs=Mg[:], start=(bi == nonempty[0]),
                                stop=(bi == nonempty[-1]))
                    if not last and (sg + 1) % (QB // SGB) == 0:
                        qd = sg // (QB // SGB)
                        if "compactag" not in ablate:
                            nc.sync.dma_start(
                                in_ccF[l][qd][:],
                                ht_sb[:][:, qd * QB:(qd + 1) * QB, :]
                                    .rearrange("p b e -> p (b e)"))
                            if "coll" not in ablate:
                                nc.gpsimd.collective_compute(
                                    "AllGather", AL.bypass,
                                    replica_groups=RG,
                                    ins=[in_ccF[l][qd].opt()],
                                    outs=[hq[l][qd].opt()])
                            continue
                        nc.sync.dma_start(
                            in_cc[l][qd][:].rearrange("p (b e) -> p b e",
                                                      e=H),
                            ht_sb[:][:, qd * QB:(qd + 1) * QB, 0:H])
                        if "coll" not in ablate:
                            nc.gpsimd.collective_compute(
                                "AllGather", AL.bypass, replica_groups=RG,
                                ins=[in_cc[l][qd].opt()],
                                outs=[hqc[l][qd].opt()])
                    if (not last and "coll" not in ablate
                            and "compactag" in ablate):
                        for (qd, pc) in exp_sched.get(sg, []):
                            expand_piece(l, qd, pc,
                                         l == 0 and qd == 0 and pc < 2)
                if (not last and "coll" not in ablate
                        and "compactag" in ablate):
                    for sgv in range(nsg, nsg + SGB):
                        for (qd, pc) in exp_sched.get(sgv, []):
                            expand_piece(l, qd, pc, False)


            poolT_sb = s_tp.tile([H, P], f32, tag="poolTs")
            nc.vector.tensor_copy(out=poolT_sb[:], in_=poolT[:])
            nc.sync.dma_start(prd_in[:], poolT_sb[:])
            nc.gpsimd.collective_compute(
                "AllReduce", AL.add, replica_groups=RG,
                ins=[prd_in.opt()], outs=[prd_out.opt()])
            poolF = s_tp.tile([H, P], f32, tag="poolF")
            nc.sync.dma_start(poolF[:], prd_out[:])
            fin = fin_ps.tile([P, C], f32, tag="fin")
            nc.tensor.matmul(fin[:], lhsT=poolF[:], rhs=wl_sb[:],
                             start=True, stop=True)
            outf = e_tp.tile([P, C], f32, tag="outf")
            nc.vector.tensor_scalar(out=outf[:], in0=fin[:],
                                    scalar1=cinv_sb[:], scalar2=None,
                                    op0=AL.mult)
            outf2 = e_tp.tile([P, C], f32, tag="outf2")
            nc.vector.tensor_tensor(out=outf2[:], in0=outf[:],
                                    in1=biasl_sb[:], op=AL.add)
            nc.sync.dma_start(out_d.ap()[:, :], outf2[:][:G, :])

    nc.compile()
    return nc


def make_in_maps(pp, weights, G, n_cores=NCORES):
    W1, b1, W2, b2, W3, b3, Wl, bl, counts = weights
    H = pp["H"]
    C = np.asarray(Wl).shape[1]
    bf = ml_dtypes.bfloat16
    iota_row = np.arange(P, dtype=np.float32)
    iota_bf = np.ascontiguousarray(np.broadcast_to(iota_row, (P, P))).astype(bf)
    iota_f = np.ascontiguousarray(np.broadcast_to(iota_row, (P, P)))
    cinv = np.ones((P, 1), np.float32)
    cinv[:G, 0] = 1.0 / np.maximum(counts, 1.0)
    shared = {
        "iota_bf": iota_bf, "iota_f": iota_f,
        "w0": np.asarray(W1, np.float32), "w1": np.asarray(W2, np.float32),
        "w2": np.asarray(W3, np.float32),
        "bias0": np.ascontiguousarray(np.broadcast_to(b1, (P, H))).astype(np.float32),
        "bias1": np.ascontiguousarray(np.broadcast_to(b2, (P, H))).astype(np.float32),
        "bias2": np.ascontiguousarray(np.broadcast_to(b3, (P, H))).astype(np.float32),
        "wl": np.asarray(Wl, np.float32),
        "biasl": np.ascontiguousarray(np.broadcast_to(bl, (P, C))).astype(np.float32),
        "cinv": cinv,
    }
    for q in range(NQ):
        shared[f"xq{q}"] = pp["xq"][q].astype(bf)
    maps = []
    for c in range(n_cores):
        m = dict(shared)
        m["eidx"] = pp["eidx16"][c]
        m["edst"] = pp["edst"][c].astype(bf)
        m["dinv"] = pp["dinv"][c]
        m["batg"] = pp["batg"][c]
        maps.append(m)
    return maps


LAST_RESULT = None
LAST_NC = None
LAST_IN_MAPS = None


def kernel(x, edge_index, batch, W1, b1, W2, b2, W3, b3, Wl, bl, **run_kwargs):
    """Full-input entry point. Shards across 8 cores, runs on HW, gathers."""
    global LAST_RESULT, LAST_NC, LAST_IN_MAPS
    from concourse.bass_utils import run_bass_kernel_spmd

    x = np.asarray(x, np.float32)
    edge_index = np.asarray(edge_index)
    batch = np.asarray(batch)
    G = G_FULL
    C = np.asarray(Wl).shape[1]

    pp = preprocess(x, edge_index, batch)
    counts = np.bincount(batch.astype(np.int64), minlength=G).astype(np.float32)
    nc = build_nc(pp, G, C)
    in_maps = make_in_maps(pp, (W1, b1, W2, b2, W3, b3, Wl, bl, counts), G)
    res = run_bass_kernel_spmd(nc, in_maps, core_ids=list(range(NCORES)),
                               **run_kwargs)
    LAST_RESULT, LAST_NC, LAST_IN_MAPS = res, nc, in_maps
    return res.results[0]["out"].astype(np.float32)
